# revision 13
# baseline (speedup 1.0000x reference)
"""Trainium2 Bass kernel: sequence-parallel multi-head self-attention block.

Computes y = proj(softmax(Q K^T / sqrt(D)) V) + b_proj for B=1, N=4096, C=768,
H=12 heads, sharded over 8 NeuronCores by sequence (512 query rows per core).

v2.5 structure (vs the 437us baseline):
  - w_qkv pre-split on the host into K/V/Q column blocks; the K projection
    (feeding the first collective) runs with only xT + half of w_k ahead of
    it on the DMA rings.  Four collectives: K-half1, V-half1, K-half2,
    V-half2, triggered as early as possible.
  - exp(softmax) split across TWO engines: ScalarE exact exp LUT, VectorE
    Schraudolph exp (int16 round of s*SCALE*128/ln2 + bias, bitcast bf16).
  - software pipeline: scores+exp run 2 k-tiles ahead of AV so the
    in-order PE queue never waits on exp.
  - score matmuls for the two heads of a pair occupy different PE row
    groups (64-dim contraction) and run concurrently; AV matmuls occupy
    different column groups (M=64 each into one shared PSUM bank halves)
    and also pack.  Softmax denominators come from M=1 ones-matmuls
    4-way column-packed into a single PSUM bank, opened by a zeroing
    matmul so `start` bank-clear semantics never bite.
  - per-head-pair normalization (reciprocal_approx_fast + rank-1 broadcast
    matmul into both partition halves) is emitted inside the NEXT pair's
    loop so the PE never idles at pair boundaries.
  - gather/weight loads are single multi-dim DMAs; next pair's loads are
    issued from inside the current pair's loop (real prefetch).
"""

import numpy as np

CORES = 8
N = 4096
S = N // CORES          # 512 query rows per core
C = 768
H = 12
D = 64
HP = H // 2             # head-pair partition tiles
CT = C // 128           # 6 contraction tiles over C
KT = N // 128           # 32 key tiles
CH = C // 2
SCALE = float(D) ** -0.5
# Schraudolph exp in bf16-via-int16: exp(s*SCALE) ~ bitcast(int16(round(
#   s*SCALE*128/ln2 + (127*128 - 5.5))))
EXP_A = SCALE * 128.0 / float(np.log(2.0))
EXP_B = 127.0 * 128.0 - 5.5

_COMPILED = None


def _build():
    from contextlib import ExitStack

    import concourse.tile as tile
    from concourse import bacc, mybir

    import ml_dtypes

    f32 = mybir.dt.float32
    f32r = mybir.dt.float32r
    bf16 = mybir.dt.bfloat16
    i16 = mybir.dt.int16
    EXP = mybir.ActivationFunctionType.Exp
    MULT = mybir.AluOpType.mult
    ADD = mybir.AluOpType.add

    nc = bacc.Bacc("TRN2", target_bir_lowering=False, debug=False,
                   num_devices=CORES)

    xT = nc.dram_tensor("xT", [C, S], f32, kind="ExternalInput")
    w_k = nc.dram_tensor("w_k", [C, C], f32, kind="ExternalInput")
    w_v = nc.dram_tensor("w_v", [C, C], f32, kind="ExternalInput")
    w_q = nc.dram_tensor("w_q", [C, C], f32, kind="ExternalInput")
    w_proj = nc.dram_tensor("w_proj", [C, C], f32, kind="ExternalInput")
    b_proj = nc.dram_tensor("b_proj", [1, C], f32, kind="ExternalInput")
    y = nc.dram_tensor("y", [S, C], f32, kind="ExternalOutput")

    bnc_k1 = nc.dram_tensor("bnc_k1", [CH, S], bf16)
    bnc_k2 = nc.dram_tensor("bnc_k2", [CH, S], bf16)
    bnc_v1 = nc.dram_tensor("bnc_v1", [S, CH], bf16)
    bnc_v2 = nc.dram_tensor("bnc_v2", [S, CH], bf16)
    gat_k1 = nc.dram_tensor("gat_k1", [CORES * CH, S], bf16,
                            addr_space="Shared")
    gat_k2 = nc.dram_tensor("gat_k2", [CORES * CH, S], bf16,
                            addr_space="Shared")
    gat_v1 = nc.dram_tensor("gat_v1", [N, CH], bf16, addr_space="Shared")
    gat_v2 = nc.dram_tensor("gat_v2", [N, CH], bf16, addr_space="Shared")

    groups = [list(range(CORES))]

    def allgather(src, dst):
        nc.gpsimd.collective_compute(
            "AllGather", mybir.AluOpType.bypass, replica_groups=groups,
            ins=[src.ap()], outs=[dst.ap()])

    with tile.TileContext(nc) as tc, ExitStack() as ctx:
        const_pool = ctx.enter_context(tc.tile_pool(name="const", bufs=1))
        qT_pool = ctx.enter_context(tc.tile_pool(name="qT", bufs=1))
        aon_pool = ctx.enter_context(tc.tile_pool(name="aon", bufs=1))
        wp_pool = ctx.enter_context(tc.tile_pool(name="wp", bufs=1))

        ones_dram = nc.inline_tensor(np.ones((128, 128), np.float32),
                                     name="ones_dram")
        ones_dram_bf = nc.inline_tensor(
            np.ones((128, 8), ml_dtypes.bfloat16), name="ones_dram_bf")
        ones_sb = const_pool.tile([128, 128], f32r, name="ones_sb")
        nc.sync.dma_start(ones_sb[:], ones_dram[:, :].bitcast(f32r))
        bp_sb = const_pool.tile([1, C], f32r, name="bp_sb")
        nc.sync.dma_start(bp_sb[:], b_proj[:, :].bitcast(f32r))
        ones_bf = const_pool.tile([128, 1], bf16, name="ones_bf")
        nc.sync.dma_start(ones_bf[:], ones_dram_bf[:, 0:1])
        zeros_bf = const_pool.tile([1, 128], bf16, name="zeros_bf")
        nc.vector.memset(zeros_bf[:], 0.0)

        qT_sb = [qT_pool.tile([128, S], bf16, name=f"qT{m}") for m in range(CT)]
        # normalized attention output per head-pair: [128 dims, S queries]
        aon_sb = [aon_pool.tile([128, S], f32r, name=f"aon{m}")
                  for m in range(CT)]
        wp_sb = wp_pool.tile([128, CT * C], f32r, name="wp_sb")

        # ---- phase 1: local qkv projection + split allgathers ----
        with tc.tile_pool(name="xw", bufs=1) as xw_pool, \
             tc.tile_pool(name="st1", bufs=1) as st1_pool, \
             tc.tile_pool(name="ps1", bufs=1, space="PSUM") as ps1_pool:
            xT_sb = xw_pool.tile([128, CT * S], f32r, name="xTs")
            nc.sync.dma_start(
                xT_sb[:].rearrange("p (k s) -> p k s", s=S),
                xT[:, :].bitcast(f32r).rearrange("(k p) s -> p k s", p=128))
            w_sbs = {}
            for nm in ("k", "v", "q"):
                w_sbs[nm] = xw_pool.tile([128, CT * C], f32r, name=f"w{nm}")
            wk_sb, wv_sb, wq_sb = w_sbs["k"], w_sbs["v"], w_sbs["q"]

            def load_w(w_sb, w_dram, c0, c1):
                # load columns [c0:c1) of every 128-row chunk in one DMA
                nc.sync.dma_start(
                    w_sb[:].rearrange("p (k c) -> p k c", c=C)[:, :, c0:c1],
                    w_dram[:, c0:c1].bitcast(f32r).rearrange(
                        "(k p) c -> p k c", p=128))

            def projT_tile(w_sb, m, dst):
                # dst[128, S] (bf16) = (w[:, 128m:128m+128]^T @ x^T)
                ps = ps1_pool.tile([128, S], f32, name="ps_p",
                                   tag="ps_p", bufs=4)
                for k in range(CT):
                    nc.tensor.matmul(
                        ps[:],
                        w_sb[:, C * k + 128 * m:C * k + 128 * (m + 1)],
                        xT_sb[:, S * k:S * (k + 1)],
                        start=(k == 0), stop=(k == CT - 1))
                nc.scalar.copy(dst[:], ps[:])

            def v_half(h, bnc):
                # V rows in natural [seq, CH] layout for column half h
                n0 = CH * h
                vst = st1_pool.tile([128, 4 * CH], bf16, name="vst",
                                    tag="vst", bufs=2)
                for mt in range(4):
                    ps = ps1_pool.tile([128, CH], f32, name="ps_v",
                                       tag="ps_v", bufs=2)
                    for k in range(CT):
                        nc.tensor.matmul(
                            ps[:],
                            xT_sb[:, S * k + 128 * mt:S * k + 128 * (mt + 1)],
                            wv_sb[:, C * k + n0:C * k + n0 + CH],
                            start=(k == 0), stop=(k == CT - 1))
                    nc.scalar.copy(vst[:, CH * mt:CH * (mt + 1)], ps[:])
                nc.sync.dma_start(
                    bnc[:, :].rearrange("(m p) c -> p m c", p=128),
                    vst[:].rearrange("p (m c) -> p m c", c=CH))

            def k_half(h, bnc):
                kst = st1_pool.tile([128, 3 * S], bf16, name="kst",
                                    tag="kst", bufs=2)
                for i, m in enumerate(range(3 * h, 3 * h + 3)):
                    projT_tile(wk_sb, m, kst[:, S * i:S * (i + 1)])
                nc.sync.dma_start(
                    bnc[:, :].rearrange("(i p) s -> p i s", p=128),
                    kst[:].rearrange("p (i s) -> p i s", s=S))

            load_w(wk_sb, w_k, 0, CH)
            k_half(0, bnc_k1)
            allgather(bnc_k1, gat_k1)
            load_w(wv_sb, w_v, 0, CH)
            v_half(0, bnc_v1)
            allgather(bnc_v1, gat_v1)
            load_w(wk_sb, w_k, CH, C)
            k_half(1, bnc_k2)
            allgather(bnc_k2, gat_k2)
            load_w(wv_sb, w_v, CH, C)
            v_half(1, bnc_v2)
            allgather(bnc_v2, gat_v2)
            load_w(wq_sb, w_q, 0, C)
            for m in range(CT):
                projT_tile(wq_sb, m, qT_sb[m])

        # ---- phase 2: attention ----
        with tc.tile_pool(name="kt", bufs=2) as kt_pool, \
             tc.tile_pool(name="vt", bufs=2) as vt_pool, \
             tc.tile_pool(name="pt", bufs=2) as pt_pool, \
             tc.tile_pool(name="nrm", bufs=2) as nrm_pool, \
             tc.tile_pool(name="sc", bufs=1, space="PSUM") as sc_pool, \
             tc.tile_pool(name="ob", bufs=1, space="PSUM") as ob_pool:

            def load_tiles(hp):
                half = hp // 3
                lhp = hp % 3
                gat_kh = (gat_k1, gat_k2)[half]
                gat_vh = (gat_v1, gat_v2)[half]
                kt = kt_pool.tile([128, N], bf16, name="kt", tag="kt", bufs=2)
                nc.sync.dma_start(
                    kt[:].rearrange("p (r s) -> p r s", s=S),
                    gat_kh[:, :].rearrange("(r c) s -> c r s",
                                           c=CH)[128 * lhp:128 * (lhp + 1)])
                vt = vt_pool.tile([128, KT * 128], bf16, name="vt", tag="vt",
                                  bufs=2)
                nc.sync.dma_start(
                    vt[:].rearrange("p (t c) -> p t c", c=128),
                    gat_vh[:, :].rearrange("(u p) c -> p u c", p=128)
                    [:, :, 128 * lhp:128 * (lhp + 1)])
                return kt, vt

            LOOK = 2
            tiles = load_tiles(0)
            pending_norm = None
            for hp in range(HP):
                kt, vt = tiles
                obp = ob_pool.tile([128, S], f32, name="obp", tag="obp",
                                   bufs=1)
                zps = ob_pool.tile([128, S], f32, name="zps", tag="zps",
                                   bufs=1)

                def scores_exp(t):
                    views = []
                    for sub in range(2):
                        sc = sc_pool.tile([128, S], f32, name=f"sc{sub}",
                                          tag=f"sc{sub}", bufs=3)
                        po = 64 * sub
                        nc.tensor.matmul(
                            sc[:],
                            kt[po:po + 64, 128 * t:128 * (t + 1)],
                            qT_sb[hp][po:po + 64, :],
                            start=True, stop=True)
                        if (t + sub) % 2 == 0:
                            pt = pt_pool.tile([128, S], bf16,
                                              name=f"ptb{sub}",
                                              tag=f"ptb{sub}", bufs=4)
                            nc.scalar.activation(pt[:], sc[:], EXP,
                                                 scale=SCALE)
                            views.append(pt[:])
                        else:
                            pt = pt_pool.tile([128, S], i16,
                                              name=f"pti{sub}",
                                              tag=f"pti{sub}", bufs=4)
                            nc.vector.tensor_scalar(pt[:], sc[:],
                                                    EXP_A, EXP_B, MULT, ADD)
                            views.append(pt[:].bitcast(bf16))
                    return views

                def av(t, views):
                    if t == 0:
                        # open both accumulator banks with a zeroing matmul
                        # (sets has_written everywhere; all real matmuls
                        # accumulate with start=False in any order)
                        for bank in (obp, zps):
                            nc.tensor.matmul(bank[:], zeros_bf[:],
                                             qT_sb[hp][0:1, :],
                                             start=True, stop=False,
                                             skip_group_check=True)
                    for sub in range(2):
                        nc.tensor.matmul(
                            obp[64 * sub:64 * (sub + 1), :],
                            vt[:, 128 * t + 64 * sub:128 * t + 64 * (sub + 1)],
                            views[sub],
                            start=False, stop=(t == KT - 1),
                            skip_group_check=True)

                def zmm(tp, views_p, views_c):
                    # 4 column-packed M=1 ones-matmuls: Z accumulators at
                    # partitions 0/32 (head A, tiles tp/tp+1) and 64/96 (B)
                    last = (tp == KT - 2)
                    for idx, vw in enumerate(
                            (views_p[0], views_c[0], views_p[1], views_c[1])):
                        pos = 32 * idx
                        nc.tensor.matmul(
                            zps[pos:pos + 1, :], ones_bf[:], vw,
                            start=False, stop=(last and idx == 3),
                            tile_position=(0, pos), skip_group_check=True)

                views_by_t = {}
                for t in range(KT):
                    views_by_t[t] = scores_exp(t)
                    if t == 6 and hp + 1 < HP:
                        tiles = load_tiles(hp + 1)  # prefetch next pair
                    if t == 10 and hp == 0:
                        # proj weights: needed only in phase 3, keep the
                        # early DMA rings clear for the collectives
                        nc.sync.dma_start(
                            wp_sb[:].rearrange("p (k c) -> p k c", c=C),
                            w_proj[:, :].bitcast(f32r).rearrange(
                                "(k p) c -> p k c", p=128))
                    if t == 4 and pending_norm is not None:
                        pending_norm()
                        pending_norm = None
                    if t >= LOOK:
                        tv = t - LOOK
                        av(tv, views_by_t[tv])
                        if tv % 2 == 1:
                            zmm(tv - 1, views_by_t[tv - 1], views_by_t[tv])
                            del views_by_t[tv - 1], views_by_t[tv]
                for t in range(KT - LOOK, KT):
                    av(t, views_by_t[t])
                    if t % 2 == 1:
                        zmm(t - 1, views_by_t[t - 1], views_by_t[t])
                        del views_by_t[t - 1], views_by_t[t]

                # evacuate accumulators now (frees PSUM for the next pair);
                # the arithmetic of the normalization is deferred into the
                # next pair's loop (pending_norm) to keep the PE dense
                aoTu = nrm_pool.tile([128, S], f32, name="aoTu", tag="aoTu",
                                     bufs=2)
                zc = nrm_pool.tile([128, S], f32, name="zc", tag="zc", bufs=2)
                nc.scalar.copy(aoTu[:], obp[:])
                nc.scalar.copy(zc[0:97, :], zps[0:97, :])

                def make_norm(hp, aoTu, zc):
                    def norm():
                        zs = nrm_pool.tile([1, 2 * S], f32, name="zs",
                                           tag="zs", bufs=2)
                        zs2 = nrm_pool.tile([1, 2 * S], f32, name="zs2",
                                            tag="zs2", bufs=2)
                        zsum = nrm_pool.tile([1, 2 * S], f32, name="zsum",
                                             tag="zsum", bufs=2)
                        rz = nrm_pool.tile([1, 2 * S], f32, name="rz",
                                           tag="rz", bufs=2)
                        # gather Z accumulator rows {0,64} and {32,96} into
                        # single-partition vectors, pair-sum, reciprocal
                        nc.sync.dma_start(zs[0:1, 0:S], zc[0:1, :])
                        nc.sync.dma_start(zs[0:1, S:2 * S], zc[64:65, :])
                        nc.sync.dma_start(zs2[0:1, 0:S], zc[32:33, :])
                        nc.sync.dma_start(zs2[0:1, S:2 * S], zc[96:97, :])
                        nc.vector.tensor_add(zsum[:], zs[:], zs2[:])
                        nc.vector.reciprocal_approx_fast(rz[:], zsum[:])
                        bc = sc_pool.tile([128, S], f32, name="bc",
                                          tag="sc0", bufs=3)
                        nc.tensor.matmul(
                            bc[0:64, :], ones_sb[0:1, 0:64].bitcast(f32),
                            rz[0:1, 0:S], start=True, stop=True,
                            skip_group_check=True)
                        nc.tensor.matmul(
                            bc[64:128, :], ones_sb[0:1, 64:128].bitcast(f32),
                            rz[0:1, S:2 * S], start=True, stop=True,
                            tile_position=(0, 64), skip_group_check=True)
                        nc.vector.tensor_mul(aon_sb[hp][:], aoTu[:], bc[:])
                    return norm

                pending_norm = make_norm(hp, aoTu, zc)
            pending_norm()

        # ---- phase 3: output projection + bias ----
        with tc.tile_pool(name="yst", bufs=2) as y_pool, \
             tc.tile_pool(name="fo", bufs=2, space="PSUM") as fo_pool:
            for mt in range(S // 128):
                yst = y_pool.tile([128, C], f32, name="yst", tag="yst", bufs=2)
                for (n0, n1) in ((0, 384), (384, 768)):
                    fo = fo_pool.tile([128, 384], f32, name="fo", tag="fo",
                                      bufs=2)
                    for k in range(CT):
                        nc.tensor.matmul(
                            fo[:],
                            aon_sb[k][:, 128 * mt:128 * (mt + 1)],
                            wp_sb[:, C * k + n0:C * k + n1],
                            start=(k == 0), stop=False)
                    nc.tensor.matmul(fo[:], ones_sb[0:1, 0:128],
                                     bp_sb[0:1, n0:n1],
                                     start=False, stop=True)
                    nc.scalar.copy(yst[:, n0:n1], fo[:])
                nc.sync.dma_start(y[128 * mt:128 * (mt + 1), :], yst[:])

    nc.compile()
    return nc


def _get_compiled():
    global _COMPILED
    if _COMPILED is None:
        _COMPILED = _build()
    return _COMPILED


def _run(inputs, trace=False):
    from concourse.bass_utils import run_bass_kernel_spmd

    nc = _get_compiled()
    x = np.asarray(inputs["x"], dtype=np.float32)
    w_qkv = np.ascontiguousarray(np.asarray(inputs["w_qkv"], dtype=np.float32))
    w_proj = np.ascontiguousarray(np.asarray(inputs["w_proj"], dtype=np.float32))
    b_proj = np.ascontiguousarray(
        np.asarray(inputs["b_proj"], dtype=np.float32).reshape(1, C))
    xT_full = np.ascontiguousarray(x[0].T)  # [C, N]
    w_q = np.ascontiguousarray(w_qkv[:, 0:C])
    w_k = np.ascontiguousarray(w_qkv[:, C:2 * C])
    w_v = np.ascontiguousarray(w_qkv[:, 2 * C:3 * C])

    in_maps = []
    for c in range(CORES):
        in_maps.append({
            "xT": np.ascontiguousarray(xT_full[:, S * c:S * (c + 1)]),
            "w_k": w_k,
            "w_v": w_v,
            "w_q": w_q,
            "w_proj": w_proj,
            "b_proj": b_proj,
        })
    res = run_bass_kernel_spmd(nc, in_maps, core_ids=list(range(CORES)),
                               trace=trace)
    out = np.concatenate([res.results[c]["y"] for c in range(CORES)], axis=0)
    return out[None, :, :].astype(np.float32), res


def kernel(**inputs) -> np.ndarray:
    out, _ = _run(inputs, trace=False)
    return out


# revision 18
# speedup vs baseline: 1.0471x; 1.0471x over previous
"""Trainium2 Bass kernel: sequence-parallel multi-head self-attention block.

Computes y = proj(softmax(Q K^T / sqrt(D)) V) + b_proj for B=1, N=4096, C=768,
H=12 heads, sharded over 8 NeuronCores by sequence (512 query rows per core).

v2.5 structure (vs the 437us baseline):
  - w_qkv pre-split on the host into K/V/Q column blocks; the K projection
    (feeding the first collective) runs with only xT + half of w_k ahead of
    it on the DMA rings.  Four collectives: K-half1, V-half1, K-half2,
    V-half2, triggered as early as possible.
  - exp(softmax) split across TWO engines: ScalarE exact exp LUT, VectorE
    Schraudolph exp (int16 round of s*SCALE*128/ln2 + bias, bitcast bf16).
  - software pipeline: scores+exp run 2 k-tiles ahead of AV so the
    in-order PE queue never waits on exp.
  - score matmuls for the two heads of a pair occupy different PE row
    groups (64-dim contraction) and run concurrently; AV matmuls occupy
    different column groups (M=64 each into one shared PSUM bank halves)
    and also pack.  Softmax denominators come from M=1 ones-matmuls
    4-way column-packed into a single PSUM bank, opened by a zeroing
    matmul so `start` bank-clear semantics never bite.
  - per-head-pair normalization (reciprocal_approx_fast + rank-1 broadcast
    matmul into both partition halves) is emitted inside the NEXT pair's
    loop so the PE never idles at pair boundaries.
  - gather/weight loads are single multi-dim DMAs; next pair's loads are
    issued from inside the current pair's loop (real prefetch).
"""

import numpy as np

CORES = 8
N = 4096
S = N // CORES          # 512 query rows per core
C = 768
H = 12
D = 64
HP = H // 2             # head-pair partition tiles
CT = C // 128           # 6 contraction tiles over C
KT = N // 128           # 32 key tiles
CH = C // 2
SCALE = float(D) ** -0.5
# Schraudolph exp in bf16-via-int16: exp(s*SCALE) ~ bitcast(int16(round(
#   s*SCALE*128/ln2 + (127*128 - 5.5))))
EXP_A = SCALE * 128.0 / float(np.log(2.0))
EXP_B = 127.0 * 128.0 - 5.5

_COMPILED = None


def _build():
    from contextlib import ExitStack

    import concourse.tile as tile
    from concourse import bacc, mybir

    import ml_dtypes

    f32 = mybir.dt.float32
    f32r = mybir.dt.float32r
    bf16 = mybir.dt.bfloat16
    i16 = mybir.dt.int16
    EXP = mybir.ActivationFunctionType.Exp
    MULT = mybir.AluOpType.mult
    ADD = mybir.AluOpType.add

    nc = bacc.Bacc("TRN2", target_bir_lowering=False, debug=False,
                   num_devices=CORES)

    xT = nc.dram_tensor("xT", [C, S], f32, kind="ExternalInput")
    w_k = nc.dram_tensor("w_k", [C, C], f32, kind="ExternalInput")
    w_v = nc.dram_tensor("w_v", [C, C], f32, kind="ExternalInput")
    w_q = nc.dram_tensor("w_q", [C, C], f32, kind="ExternalInput")
    w_proj = nc.dram_tensor("w_proj", [C, C], f32, kind="ExternalInput")
    b_proj = nc.dram_tensor("b_proj", [1, C], f32, kind="ExternalInput")
    y = nc.dram_tensor("y", [S, C], f32, kind="ExternalOutput")

    # merged per-half bounce: rows 0-383 = K^T half, rows 384-895 = V half
    # (V's [S, CH] rows padded to 512-wide so the gather APs stay clean)
    BR = CH + S
    bnc_a = nc.dram_tensor("bnc_a", [BR, S], bf16)
    bnc_b = nc.dram_tensor("bnc_b", [BR, S], bf16)
    gat_a = nc.dram_tensor("gat_a", [CORES * BR, S], bf16,
                           addr_space="Shared")
    gat_b = nc.dram_tensor("gat_b", [CORES * BR, S], bf16,
                           addr_space="Shared")

    groups = [list(range(CORES))]

    def allgather(src, dst):
        nc.gpsimd.collective_compute(
            "AllGather", mybir.AluOpType.bypass, replica_groups=groups,
            ins=[src.ap()], outs=[dst.ap()])

    with tile.TileContext(nc) as tc, ExitStack() as ctx:
        const_pool = ctx.enter_context(tc.tile_pool(name="const", bufs=1))
        qT_pool = ctx.enter_context(tc.tile_pool(name="qT", bufs=1))
        aon_pool = ctx.enter_context(tc.tile_pool(name="aon", bufs=1))
        wp_pool = ctx.enter_context(tc.tile_pool(name="wp", bufs=1))

        ones_dram = nc.inline_tensor(np.ones((128, 128), np.float32),
                                     name="ones_dram")
        ones_dram_bf = nc.inline_tensor(
            np.ones((128, 8), ml_dtypes.bfloat16), name="ones_dram_bf")
        ones_sb = const_pool.tile([128, 128], f32r, name="ones_sb")
        nc.sync.dma_start(ones_sb[:], ones_dram[:, :].bitcast(f32r))
        bp_sb = const_pool.tile([1, C], f32r, name="bp_sb")
        nc.sync.dma_start(bp_sb[:], b_proj[:, :].bitcast(f32r))
        ones_bf = const_pool.tile([128, 1], bf16, name="ones_bf")
        nc.sync.dma_start(ones_bf[:], ones_dram_bf[:, 0:1])
        zeros_bf = const_pool.tile([1, 128], bf16, name="zeros_bf")
        nc.vector.memset(zeros_bf[:], 0.0)

        qT_sb = [qT_pool.tile([128, S], bf16, name=f"qT{m}") for m in range(CT)]
        # normalized attention output per head-pair: [128 dims, S queries]
        aon_sb = [aon_pool.tile([128, S], f32r, name=f"aon{m}")
                  for m in range(CT)]
        wp_sb = wp_pool.tile([128, CT * C], f32r, name="wp_sb")

        # ---- phase 1: local qkv projection + split allgathers ----
        with tc.tile_pool(name="xw", bufs=1) as xw_pool, \
             tc.tile_pool(name="st1", bufs=1) as st1_pool, \
             tc.tile_pool(name="ps1", bufs=1, space="PSUM") as ps1_pool:
            xT_sb = xw_pool.tile([128, CT * S], f32r, name="xTs")
            nc.sync.dma_start(
                xT_sb[:].rearrange("p (k s) -> p k s", s=S),
                xT[:, :].bitcast(f32r).rearrange("(k p) s -> p k s", p=128))
            w_sbs = {}
            for nm in ("k", "v", "q"):
                w_sbs[nm] = xw_pool.tile([128, CT * C], f32r, name=f"w{nm}")
            wk_sb, wv_sb, wq_sb = w_sbs["k"], w_sbs["v"], w_sbs["q"]

            def load_w(w_sb, w_dram, c0, c1):
                # load columns [c0:c1) of every 128-row chunk in one DMA
                nc.sync.dma_start(
                    w_sb[:].rearrange("p (k c) -> p k c", c=C)[:, :, c0:c1],
                    w_dram[:, c0:c1].bitcast(f32r).rearrange(
                        "(k p) c -> p k c", p=128))

            def projT_tile(w_sb, m, dst):
                # dst[128, S] (bf16) = (w[:, 128m:128m+128]^T @ x^T)
                ps = ps1_pool.tile([128, S], f32, name="ps_p",
                                   tag="ps_p", bufs=4)
                for k in range(CT):
                    nc.tensor.matmul(
                        ps[:],
                        w_sb[:, C * k + 128 * m:C * k + 128 * (m + 1)],
                        xT_sb[:, S * k:S * (k + 1)],
                        start=(k == 0), stop=(k == CT - 1))
                nc.scalar.copy(dst[:], ps[:])

            def v_half(h, bnc):
                # V rows in natural [seq, CH] layout for column half h,
                # into bounce rows 384+ (512-wide rows, cols 384: unused)
                n0 = CH * h
                vst = st1_pool.tile([128, 4 * CH], bf16, name="vst",
                                    tag="vst", bufs=2)
                for mt in range(4):
                    ps = ps1_pool.tile([128, CH], f32, name="ps_v",
                                       tag="ps_v", bufs=2)
                    for k in range(CT):
                        nc.tensor.matmul(
                            ps[:],
                            xT_sb[:, S * k + 128 * mt:S * k + 128 * (mt + 1)],
                            wv_sb[:, C * k + n0:C * k + n0 + CH],
                            start=(k == 0), stop=(k == CT - 1))
                    nc.scalar.copy(vst[:, CH * mt:CH * (mt + 1)], ps[:])
                nc.sync.dma_start(
                    bnc[CH:, 0:CH].rearrange("(m p) c -> p m c", p=128),
                    vst[:].rearrange("p (m c) -> p m c", c=CH))

            def k_half(h, bnc):
                kst = st1_pool.tile([128, 3 * S], bf16, name="kst",
                                    tag="kst", bufs=2)
                for i, m in enumerate(range(3 * h, 3 * h + 3)):
                    projT_tile(wk_sb, m, kst[:, S * i:S * (i + 1)])
                nc.sync.dma_start(
                    bnc[0:CH, :].rearrange("(i p) s -> p i s", p=128),
                    kst[:].rearrange("p (i s) -> p i s", s=S))

            load_w(wk_sb, w_k, 0, CH)
            k_half(0, bnc_a)
            load_w(wv_sb, w_v, 0, CH)
            v_half(0, bnc_a)
            allgather(bnc_a, gat_a)
            load_w(wk_sb, w_k, CH, C)
            k_half(1, bnc_b)
            load_w(wv_sb, w_v, CH, C)
            v_half(1, bnc_b)
            allgather(bnc_b, gat_b)
            load_w(wq_sb, w_q, 0, C)
            for m in range(CT):
                projT_tile(wq_sb, m, qT_sb[m])

        # ---- phase 2: attention ----
        with tc.tile_pool(name="kt", bufs=2) as kt_pool, \
             tc.tile_pool(name="vt", bufs=2) as vt_pool, \
             tc.tile_pool(name="pt", bufs=2) as pt_pool, \
             tc.tile_pool(name="nrm", bufs=2) as nrm_pool, \
             tc.tile_pool(name="sc", bufs=1, space="PSUM") as sc_pool, \
             tc.tile_pool(name="ob", bufs=1, space="PSUM") as ob_pool:

            def load_tiles(hp):
                half = hp // 3
                lhp = hp % 3
                gat_h = (gat_a, gat_b)[half]
                kt = kt_pool.tile([128, N], bf16, name="kt", tag="kt", bufs=2)
                nc.sync.dma_start(
                    kt[:].rearrange("p (r s) -> p r s", s=S),
                    gat_h[:, :].rearrange("(r c) s -> c r s",
                                          c=BR)[128 * lhp:128 * (lhp + 1)])
                vt = vt_pool.tile([128, KT * 128], bf16, name="vt", tag="vt",
                                  bufs=2)
                for tl in range(4):
                    nc.sync.dma_start(
                        vt[:].rearrange("p (r t c) -> p t r c",
                                        t=4, c=128)[:, tl],
                        gat_h[:, :].rearrange("(r v) s -> v r s", v=BR)
                        [CH + 128 * tl:CH + 128 * (tl + 1), :,
                         128 * lhp:128 * (lhp + 1)])
                return kt, vt

            LOOK = 2
            tiles = load_tiles(0)
            pending_norm = None
            for hp in range(HP):
                kt, vt = tiles
                obp = ob_pool.tile([128, S], f32, name="obp", tag="obp",
                                   bufs=1)
                zps = ob_pool.tile([128, S], f32, name="zps", tag="zps",
                                   bufs=1)

                def scores_exp(t):
                    views = []
                    for sub in range(2):
                        sc = sc_pool.tile([128, S], f32, name=f"sc{sub}",
                                          tag=f"sc{sub}", bufs=3)
                        po = 64 * sub
                        nc.tensor.matmul(
                            sc[:],
                            kt[po:po + 64, 128 * t:128 * (t + 1)],
                            qT_sb[hp][po:po + 64, :],
                            start=True, stop=True)
                        if (t + sub) % 2 == 0:
                            pt = pt_pool.tile([128, S], bf16,
                                              name=f"ptb{sub}",
                                              tag=f"ptb{sub}", bufs=4)
                            nc.scalar.activation(pt[:], sc[:], EXP,
                                                 scale=SCALE)
                            views.append(pt[:])
                        else:
                            pt = pt_pool.tile([128, S], i16,
                                              name=f"pti{sub}",
                                              tag=f"pti{sub}", bufs=4)
                            nc.vector.tensor_scalar(pt[:], sc[:],
                                                    EXP_A, EXP_B, MULT, ADD)
                            views.append(pt[:].bitcast(bf16))
                    return views

                def av(t, views):
                    if t == 0:
                        # open both accumulator banks with a zeroing matmul
                        # (sets has_written everywhere; all real matmuls
                        # accumulate with start=False in any order)
                        for bank in (obp, zps):
                            nc.tensor.matmul(bank[:], zeros_bf[:],
                                             qT_sb[hp][0:1, :],
                                             start=True, stop=False,
                                             skip_group_check=True)
                    for sub in range(2):
                        nc.tensor.matmul(
                            obp[64 * sub:64 * (sub + 1), :],
                            vt[:, 128 * t + 64 * sub:128 * t + 64 * (sub + 1)],
                            views[sub],
                            start=False, stop=(t == KT - 1),
                            skip_group_check=True)

                def zmm(tp, views_p, views_c):
                    # 4 column-packed M=1 ones-matmuls: Z accumulators at
                    # partitions 0/32 (head A, tiles tp/tp+1) and 64/96 (B)
                    last = (tp == KT - 2)
                    for idx, vw in enumerate(
                            (views_p[0], views_c[0], views_p[1], views_c[1])):
                        pos = 32 * idx
                        nc.tensor.matmul(
                            zps[pos:pos + 1, :], ones_bf[:], vw,
                            start=False, stop=(last and idx == 3),
                            tile_position=(0, pos), skip_group_check=True)

                views_by_t = {}
                for t in range(KT):
                    views_by_t[t] = scores_exp(t)
                    if t == 6 and hp + 1 < HP:
                        tiles = load_tiles(hp + 1)  # prefetch next pair
                    if t == 10 and hp == 0:
                        # proj weights: needed only in phase 3, keep the
                        # early DMA rings clear for the collectives
                        nc.sync.dma_start(
                            wp_sb[:].rearrange("p (k c) -> p k c", c=C),
                            w_proj[:, :].bitcast(f32r).rearrange(
                                "(k p) c -> p k c", p=128))
                    if t == 8 and pending_norm is not None:
                        pending_norm()
                        pending_norm = None
                    if t >= LOOK:
                        tv = t - LOOK
                        av(tv, views_by_t[tv])
                        if tv % 2 == 1:
                            zmm(tv - 1, views_by_t[tv - 1], views_by_t[tv])
                            del views_by_t[tv - 1], views_by_t[tv]
                for t in range(KT - LOOK, KT):
                    av(t, views_by_t[t])
                    if t % 2 == 1:
                        zmm(t - 1, views_by_t[t - 1], views_by_t[t])
                        del views_by_t[t - 1], views_by_t[t]

                # evacuate accumulators now (frees PSUM for the next pair);
                # the arithmetic of the normalization is deferred into the
                # next pair's loop (pending_norm) to keep the PE dense
                aoTu = nrm_pool.tile([128, S], f32, name="aoTu", tag="aoTu",
                                     bufs=2)
                zc = nrm_pool.tile([128, S], f32, name="zc", tag="zc", bufs=2)
                nc.scalar.copy(aoTu[:], obp[:])
                nc.scalar.copy(zc[0:97, :], zps[0:97, :])

                def make_norm(hp, aoTu, zc):
                    def norm():
                        zs = nrm_pool.tile([1, 2 * S], f32, name="zs",
                                           tag="zs", bufs=2)
                        zs2 = nrm_pool.tile([1, 2 * S], f32, name="zs2",
                                            tag="zs2", bufs=2)
                        zsum = nrm_pool.tile([1, 2 * S], f32, name="zsum",
                                             tag="zsum", bufs=2)
                        rz = nrm_pool.tile([1, 2 * S], f32, name="rz",
                                           tag="rz", bufs=2)
                        # gather Z accumulator rows {0,64} and {32,96} into
                        # single-partition vectors, pair-sum, reciprocal
                        # (scalar queue: keep these off the busy sync queue)
                        nc.scalar.dma_start(zs[0:1, 0:S], zc[0:1, :])
                        nc.scalar.dma_start(zs[0:1, S:2 * S], zc[64:65, :])
                        nc.scalar.dma_start(zs2[0:1, 0:S], zc[32:33, :])
                        nc.scalar.dma_start(zs2[0:1, S:2 * S], zc[96:97, :])
                        nc.vector.tensor_add(zsum[:], zs[:], zs2[:])
                        nc.vector.reciprocal_approx_fast(rz[:], zsum[:])
                        bc = sc_pool.tile([128, S], f32, name="bc",
                                          tag="sc0", bufs=3)
                        nc.tensor.matmul(
                            bc[0:64, :], ones_sb[0:1, 0:64].bitcast(f32),
                            rz[0:1, 0:S], start=True, stop=True,
                            skip_group_check=True)
                        nc.tensor.matmul(
                            bc[64:128, :], ones_sb[0:1, 64:128].bitcast(f32),
                            rz[0:1, S:2 * S], start=True, stop=True,
                            tile_position=(0, 64), skip_group_check=True)
                        nc.vector.tensor_mul(aon_sb[hp][:], aoTu[:], bc[:])
                    return norm

                pending_norm = make_norm(hp, aoTu, zc)
            pending_norm()

        # ---- phase 3: output projection + bias ----
        with tc.tile_pool(name="yst", bufs=2) as y_pool, \
             tc.tile_pool(name="fo", bufs=2, space="PSUM") as fo_pool:
            for mt in range(S // 128):
                yst = y_pool.tile([128, C], f32, name="yst", tag="yst", bufs=2)
                for (n0, n1) in ((0, 384), (384, 768)):
                    fo = fo_pool.tile([128, 384], f32, name="fo", tag="fo",
                                      bufs=2)
                    for k in range(CT):
                        nc.tensor.matmul(
                            fo[:],
                            aon_sb[k][:, 128 * mt:128 * (mt + 1)],
                            wp_sb[:, C * k + n0:C * k + n1],
                            start=(k == 0), stop=False)
                    nc.tensor.matmul(fo[:], ones_sb[0:1, 0:128],
                                     bp_sb[0:1, n0:n1],
                                     start=False, stop=True)
                    nc.scalar.copy(yst[:, n0:n1], fo[:])
                nc.sync.dma_start(y[128 * mt:128 * (mt + 1), :], yst[:])

    nc.compile()
    return nc


def _get_compiled():
    global _COMPILED
    if _COMPILED is None:
        _COMPILED = _build()
    return _COMPILED


def _run(inputs, trace=False):
    from concourse.bass_utils import run_bass_kernel_spmd

    nc = _get_compiled()
    x = np.asarray(inputs["x"], dtype=np.float32)
    w_qkv = np.ascontiguousarray(np.asarray(inputs["w_qkv"], dtype=np.float32))
    w_proj = np.ascontiguousarray(np.asarray(inputs["w_proj"], dtype=np.float32))
    b_proj = np.ascontiguousarray(
        np.asarray(inputs["b_proj"], dtype=np.float32).reshape(1, C))
    xT_full = np.ascontiguousarray(x[0].T)  # [C, N]
    w_q = np.ascontiguousarray(w_qkv[:, 0:C])
    w_k = np.ascontiguousarray(w_qkv[:, C:2 * C])
    w_v = np.ascontiguousarray(w_qkv[:, 2 * C:3 * C])

    in_maps = []
    for c in range(CORES):
        in_maps.append({
            "xT": np.ascontiguousarray(xT_full[:, S * c:S * (c + 1)]),
            "w_k": w_k,
            "w_v": w_v,
            "w_q": w_q,
            "w_proj": w_proj,
            "b_proj": b_proj,
        })
    res = run_bass_kernel_spmd(nc, in_maps, core_ids=list(range(CORES)),
                               trace=trace)
    out = np.concatenate([res.results[c]["y"] for c in range(CORES)], axis=0)
    return out[None, :, :].astype(np.float32), res


def kernel(**inputs) -> np.ndarray:
    out, _ = _run(inputs, trace=False)
    return out


# revision 26
# speedup vs baseline: 1.1780x; 1.1249x over previous
"""Trainium2 Bass kernel: sequence-parallel multi-head self-attention block.

Computes y = proj(softmax(Q K^T / sqrt(D)) V) + b_proj for B=1, N=4096, C=768,
H=12 heads, sharded over 8 NeuronCores by sequence (512 query rows per core).

v2.5 structure (vs the 437us baseline):
  - w_qkv pre-split on the host into K/V/Q column blocks; the K projection
    (feeding the first collective) runs with only xT + half of w_k ahead of
    it on the DMA rings.  Four collectives: K-half1, V-half1, K-half2,
    V-half2, triggered as early as possible.
  - exp(softmax) split across TWO engines: ScalarE exact exp LUT, VectorE
    Schraudolph exp (int16 round of s*SCALE*128/ln2 + bias, bitcast bf16).
  - software pipeline: scores+exp run 2 k-tiles ahead of AV so the
    in-order PE queue never waits on exp.
  - score matmuls for the two heads of a pair occupy different PE row
    groups (64-dim contraction) and run concurrently; AV matmuls occupy
    different column groups (M=64 each into one shared PSUM bank halves)
    and also pack.  Softmax denominators come from M=1 ones-matmuls
    4-way column-packed into a single PSUM bank, opened by a zeroing
    matmul so `start` bank-clear semantics never bite.
  - per-head-pair normalization (reciprocal_approx_fast + rank-1 broadcast
    matmul into both partition halves) is emitted inside the NEXT pair's
    loop so the PE never idles at pair boundaries.
  - gather/weight loads are single multi-dim DMAs; next pair's loads are
    issued from inside the current pair's loop (real prefetch).
"""

import numpy as np

CORES = 8
N = 4096
S = N // CORES          # 512 query rows per core
C = 768
H = 12
D = 64
HP = H // 2             # head-pair partition tiles
CT = C // 128           # 6 contraction tiles over C
KT = N // 128           # 32 key tiles
CH = C // 2
SCALE = float(D) ** -0.5
# Schraudolph exp in bf16-via-int16: exp(s*SCALE) ~ bitcast(int16(round(
#   s*SCALE*128/ln2 + (127*128 - 5.5))))
EXP_A = SCALE * 128.0 / float(np.log(2.0))
EXP_B = 127.0 * 128.0 - 5.5

_COMPILED = None


def _build():
    from contextlib import ExitStack

    import concourse.tile as tile
    from concourse import bacc, mybir

    import ml_dtypes

    f32 = mybir.dt.float32
    f32r = mybir.dt.float32r
    bf16 = mybir.dt.bfloat16
    i16 = mybir.dt.int16
    EXP = mybir.ActivationFunctionType.Exp
    MULT = mybir.AluOpType.mult
    ADD = mybir.AluOpType.add

    nc = bacc.Bacc("TRN2", target_bir_lowering=False, debug=False,
                   num_devices=CORES)

    xT = nc.dram_tensor("xT", [C, S], f32, kind="ExternalInput")
    w_k = nc.dram_tensor("w_k", [C, C], f32, kind="ExternalInput")
    w_v = nc.dram_tensor("w_v", [C, C], f32, kind="ExternalInput")
    w_q = nc.dram_tensor("w_q", [C, C], f32, kind="ExternalInput")
    w_proj = nc.dram_tensor("w_proj", [C, C], f32, kind="ExternalInput")
    b_bcast = nc.dram_tensor("b_bcast", [128, C], f32, kind="ExternalInput")
    y = nc.dram_tensor("y", [S, C], f32, kind="ExternalOutput")

    # merged per-half bounce: rows 0-383 = K^T half, rows 384-895 = V half
    # (V's [S, CH] rows padded to 512-wide so the gather APs stay clean)
    BR = CH + S
    bnc_a = nc.dram_tensor("bnc_a", [BR, S], bf16)
    bnc_b = nc.dram_tensor("bnc_b", [BR, S], bf16)
    gat_a = nc.dram_tensor("gat_a", [CORES * BR, S], bf16,
                           addr_space="Shared")
    gat_b = nc.dram_tensor("gat_b", [CORES * BR, S], bf16,
                           addr_space="Shared")

    # tiny dummy collective fired first: absorbs the ~50us first-collective
    # ncfw warmup while phase 1 computes
    bnc_w = nc.dram_tensor("bnc_w", [1, 16], bf16)
    gat_w = nc.dram_tensor("gat_w", [CORES, 16], bf16, addr_space="Shared")

    groups = [list(range(CORES))]

    def allgather(src, dst):
        nc.gpsimd.collective_compute(
            "AllGather", mybir.AluOpType.bypass, replica_groups=groups,
            ins=[src.ap()], outs=[dst.ap()])

    with tile.TileContext(nc) as tc, ExitStack() as ctx:
        const_pool = ctx.enter_context(tc.tile_pool(name="const", bufs=1))
        qT_pool = ctx.enter_context(tc.tile_pool(name="qT", bufs=1))
        aon_pool = ctx.enter_context(tc.tile_pool(name="aon", bufs=1))
        wp_pool = ctx.enter_context(tc.tile_pool(name="wp", bufs=1))

        ones_dram = nc.inline_tensor(np.ones((128, 128), np.float32),
                                     name="ones_dram")
        ones_dram_bf = nc.inline_tensor(
            np.ones((128, 8), ml_dtypes.bfloat16), name="ones_dram_bf")
        # kick the warmup collective before anything else
        wseed = const_pool.tile([1, 16], bf16, name="wseed")
        nc.vector.memset(wseed[:], 0.0)
        nc.sync.dma_start(bnc_w[:, :], wseed[:])
        allgather(bnc_w, gat_w)

        ones_sb = const_pool.tile([128, 128], f32r, name="ones_sb")
        nc.sync.dma_start(ones_sb[:], ones_dram[:, :].bitcast(f32r))
        bb_sb = const_pool.tile([128, C], f32, name="bb_sb")
        nc.sync.dma_start(bb_sb[:], b_bcast[:, :])
        ones_bf = const_pool.tile([128, 1], bf16, name="ones_bf")
        nc.sync.dma_start(ones_bf[:], ones_dram_bf[:, 0:1])
        zeros_bf = const_pool.tile([1, 128], bf16, name="zeros_bf")
        nc.vector.memset(zeros_bf[:], 0.0)

        qT_sb = [qT_pool.tile([128, S], bf16, name=f"qT{m}") for m in range(CT)]
        # normalized attention output per head-pair: [128 dims, S queries]
        aon_sb = [aon_pool.tile([128, S], f32r, name=f"aon{m}")
                  for m in range(CT)]
        wp_sb = wp_pool.tile([128, CT * C], f32r, name="wp_sb")

        # ---- phase 1: local qkv projection + split allgathers ----
        with tc.tile_pool(name="xw", bufs=1) as xw_pool, \
             tc.tile_pool(name="st1", bufs=1) as st1_pool, \
             tc.tile_pool(name="ps1", bufs=1, space="PSUM") as ps1_pool:
            xT_sb = xw_pool.tile([128, CT * S], f32r, name="xTs")
            nc.sync.dma_start(
                xT_sb[:].rearrange("p (k s) -> p k s", s=S),
                xT[:, :].bitcast(f32r).rearrange("(k p) s -> p k s", p=128))
            w_sbs = {}
            for nm in ("k", "v", "q"):
                w_sbs[nm] = xw_pool.tile([128, CT * C], f32r, name=f"w{nm}")
            wk_sb, wv_sb, wq_sb = w_sbs["k"], w_sbs["v"], w_sbs["q"]

            def load_w(w_sb, w_dram, c0, c1):
                # load columns [c0:c1) of every 128-row chunk in one DMA
                nc.sync.dma_start(
                    w_sb[:].rearrange("p (k c) -> p k c", c=C)[:, :, c0:c1],
                    w_dram[:, c0:c1].bitcast(f32r).rearrange(
                        "(k p) c -> p k c", p=128))

            def projT_tile(w_sb, m, dst):
                # dst[128, S] (bf16) = (w[:, 128m:128m+128]^T @ x^T)
                ps = ps1_pool.tile([128, S], f32, name="ps_p",
                                   tag="ps_p", bufs=4)
                for k in range(CT):
                    nc.tensor.matmul(
                        ps[:],
                        w_sb[:, C * k + 128 * m:C * k + 128 * (m + 1)],
                        xT_sb[:, S * k:S * (k + 1)],
                        start=(k == 0), stop=(k == CT - 1))
                nc.scalar.copy(dst[:], ps[:])

            def v_half(h, bnc):
                # V rows in natural [seq, CH] layout for column half h,
                # into bounce rows 384+ (512-wide rows, cols 384: unused)
                n0 = CH * h
                vst = st1_pool.tile([128, 4 * CH], bf16, name="vst",
                                    tag="vst", bufs=2)
                for mt in range(4):
                    ps = ps1_pool.tile([128, CH], f32, name="ps_v",
                                       tag="ps_v", bufs=2)
                    for k in range(CT):
                        nc.tensor.matmul(
                            ps[:],
                            xT_sb[:, S * k + 128 * mt:S * k + 128 * (mt + 1)],
                            wv_sb[:, C * k + n0:C * k + n0 + CH],
                            start=(k == 0), stop=(k == CT - 1))
                    nc.scalar.copy(vst[:, CH * mt:CH * (mt + 1)], ps[:])
                nc.sync.dma_start(
                    bnc[CH:, 0:CH].rearrange("(m p) c -> p m c", p=128),
                    vst[:].rearrange("p (m c) -> p m c", c=CH))

            def k_half(h, bnc):
                kst = st1_pool.tile([128, 3 * S], bf16, name="kst",
                                    tag="kst", bufs=2)
                for i, m in enumerate(range(3 * h, 3 * h + 3)):
                    projT_tile(wk_sb, m, kst[:, S * i:S * (i + 1)])
                nc.sync.dma_start(
                    bnc[0:CH, :].rearrange("(i p) s -> p i s", p=128),
                    kst[:].rearrange("p (i s) -> p i s", s=S))

            load_w(wk_sb, w_k, 0, CH)
            k_half(0, bnc_a)
            load_w(wv_sb, w_v, 0, CH)
            v_half(0, bnc_a)
            allgather(bnc_a, gat_a)
            load_w(wk_sb, w_k, CH, C)
            k_half(1, bnc_b)
            load_w(wv_sb, w_v, CH, C)
            v_half(1, bnc_b)
            allgather(bnc_b, gat_b)
            load_w(wq_sb, w_q, 0, C)
            for m in range(CT):
                projT_tile(wq_sb, m, qT_sb[m])

        # ---- phase 2: attention ----
        with tc.tile_pool(name="kt", bufs=2) as kt_pool, \
             tc.tile_pool(name="vt", bufs=2) as vt_pool, \
             tc.tile_pool(name="pt", bufs=2) as pt_pool, \
             tc.tile_pool(name="nrm", bufs=2) as nrm_pool, \
             tc.tile_pool(name="sc", bufs=1, space="PSUM") as sc_pool, \
             tc.tile_pool(name="ob", bufs=1, space="PSUM") as ob_pool:

            def load_tiles(hp):
                half = hp // 3
                lhp = hp % 3
                gat_h = (gat_a, gat_b)[half]
                kt = kt_pool.tile([128, N], bf16, name="kt", tag="kt", bufs=2)
                nc.sync.dma_start(
                    kt[:].rearrange("p (r s) -> p r s", s=S),
                    gat_h[:, :].rearrange("(r c) s -> c r s",
                                          c=BR)[128 * lhp:128 * (lhp + 1)])
                vt = vt_pool.tile([128, KT * 128], bf16, name="vt", tag="vt",
                                  bufs=2)
                for tl in range(4):
                    nc.sync.dma_start(
                        vt[:].rearrange("p (r t c) -> p t r c",
                                        t=4, c=128)[:, tl],
                        gat_h[:, :].rearrange("(r v) s -> v r s", v=BR)
                        [CH + 128 * tl:CH + 128 * (tl + 1), :,
                         128 * lhp:128 * (lhp + 1)])
                return kt, vt

            LOOK = 2
            tiles = load_tiles(0)
            pending_norm = None
            for hp in range(HP):
                kt, vt = tiles
                obp = ob_pool.tile([128, S], f32, name="obp", tag="obp",
                                   bufs=1)
                zps = ob_pool.tile([128, S], f32, name="zps", tag="zps",
                                   bufs=1)

                def scores_exp(t):
                    views = []
                    for sub in range(2):
                        sc = sc_pool.tile([128, S], f32, name=f"sc{sub}",
                                          tag=f"sc{sub}", bufs=3)
                        po = 64 * sub
                        nc.tensor.matmul(
                            sc[:],
                            kt[po:po + 64, 128 * t:128 * (t + 1)],
                            qT_sb[hp][po:po + 64, :],
                            start=True, stop=True)
                        if (t + sub) % 2 == 0:
                            pt = pt_pool.tile([128, S], bf16,
                                              name=f"ptb{sub}",
                                              tag=f"ptb{sub}", bufs=4)
                            nc.scalar.activation(pt[:], sc[:], EXP,
                                                 scale=SCALE)
                            views.append(pt[:])
                        else:
                            pt = pt_pool.tile([128, S], i16,
                                              name=f"pti{sub}",
                                              tag=f"pti{sub}", bufs=4)
                            nc.vector.tensor_scalar(pt[:], sc[:],
                                                    EXP_A, EXP_B, MULT, ADD)
                            views.append(pt[:].bitcast(bf16))
                    return views

                def av(t, views):
                    if t == 0:
                        # open both accumulator banks with a zeroing matmul
                        # (sets has_written everywhere; all real matmuls
                        # accumulate with start=False in any order)
                        for bank in (obp, zps):
                            nc.tensor.matmul(bank[:], zeros_bf[:],
                                             qT_sb[hp][0:1, :],
                                             start=True, stop=False,
                                             skip_group_check=True)
                    for sub in range(2):
                        nc.tensor.matmul(
                            obp[64 * sub:64 * (sub + 1), :],
                            vt[:, 128 * t + 64 * sub:128 * t + 64 * (sub + 1)],
                            views[sub],
                            start=False, stop=(t == KT - 1),
                            skip_group_check=True)

                def zmm(tp, views_p, views_c):
                    # 4 column-packed M=1 ones-matmuls: Z accumulators at
                    # partitions 0/32 (head A, tiles tp/tp+1) and 64/96 (B)
                    last = (tp == KT - 2)
                    for idx, vw in enumerate(
                            (views_p[0], views_c[0], views_p[1], views_c[1])):
                        pos = 32 * idx
                        nc.tensor.matmul(
                            zps[pos:pos + 1, :], ones_bf[:], vw,
                            start=False, stop=(last and idx == 3),
                            tile_position=(0, pos), skip_group_check=True)

                views_by_t = {}
                for t in range(KT):
                    views_by_t[t] = scores_exp(t)
                    if t == 6 and hp + 1 < HP:
                        tiles = load_tiles(hp + 1)  # prefetch next pair
                    if t == 10 and hp == 0:
                        # proj weights: needed only in phase 3, keep the
                        # early DMA rings clear for the collectives
                        nc.sync.dma_start(
                            wp_sb[:].rearrange("p (k c) -> p k c", c=C),
                            w_proj[:, :].bitcast(f32r).rearrange(
                                "(k p) c -> p k c", p=128))
                    if t == 8 and pending_norm is not None:
                        pending_norm()
                        pending_norm = None
                    if t >= LOOK:
                        tv = t - LOOK
                        av(tv, views_by_t[tv])
                        if tv % 2 == 1:
                            zmm(tv - 1, views_by_t[tv - 1], views_by_t[tv])
                            del views_by_t[tv - 1], views_by_t[tv]
                for t in range(KT - LOOK, KT):
                    av(t, views_by_t[t])
                    if t % 2 == 1:
                        zmm(t - 1, views_by_t[t - 1], views_by_t[t])
                        del views_by_t[t - 1], views_by_t[t]

                # evacuate accumulators now (frees PSUM for the next pair);
                # the arithmetic of the normalization is deferred into the
                # next pair's loop (pending_norm) to keep the PE dense
                aoTu = nrm_pool.tile([128, S], f32, name="aoTu", tag="aoTu",
                                     bufs=2)
                zc = nrm_pool.tile([128, S], f32, name="zc", tag="zc", bufs=2)
                nc.scalar.copy(aoTu[:], obp[:])
                nc.scalar.copy(zc[0:97, :], zps[0:97, :])

                def make_norm(hp, aoTu, zc):
                    def norm():
                        zs = nrm_pool.tile([1, 2 * S], f32, name="zs",
                                           tag="zs", bufs=2)
                        zs2 = nrm_pool.tile([1, 2 * S], f32, name="zs2",
                                            tag="zs2", bufs=2)
                        zsum = nrm_pool.tile([1, 2 * S], f32, name="zsum",
                                             tag="zsum", bufs=2)
                        rz = nrm_pool.tile([1, 2 * S], f32, name="rz",
                                           tag="rz", bufs=2)
                        # gather Z accumulator rows {0,64} and {32,96} into
                        # single-partition vectors, pair-sum, reciprocal
                        # (spread over queues so the 4 issues overlap)
                        nc.scalar.dma_start(zs[0:1, 0:S], zc[0:1, :])
                        nc.gpsimd.dma_start(zs[0:1, S:2 * S], zc[64:65, :])
                        nc.scalar.dma_start(zs2[0:1, 0:S], zc[32:33, :])
                        nc.gpsimd.dma_start(zs2[0:1, S:2 * S], zc[96:97, :])
                        nc.vector.tensor_add(zsum[:], zs[:], zs2[:])
                        nc.vector.reciprocal_approx_fast(rz[:], zsum[:])
                        bc = sc_pool.tile([128, S], f32, name="bc",
                                          tag="sc0", bufs=3)
                        nc.tensor.matmul(
                            bc[0:64, :], ones_sb[0:1, 0:64].bitcast(f32),
                            rz[0:1, 0:S], start=True, stop=True,
                            skip_group_check=True)
                        nc.tensor.matmul(
                            bc[64:128, :], ones_sb[0:1, 64:128].bitcast(f32),
                            rz[0:1, S:2 * S], start=True, stop=True,
                            tile_position=(0, 64), skip_group_check=True)
                        nc.vector.tensor_mul(aon_sb[hp][:], aoTu[:], bc[:])
                    return norm

                pending_norm = make_norm(hp, aoTu, zc)
            pending_norm()

        # ---- phase 3: output projection + bias ----
        with tc.tile_pool(name="yst", bufs=2) as y_pool, \
             tc.tile_pool(name="fo", bufs=2, space="PSUM") as fo_pool:
            for mt in range(S // 128):
                yst = y_pool.tile([128, C], f32, name="yst", tag="yst", bufs=2)
                for (n0, n1) in ((0, 384), (384, 768)):
                    fo = fo_pool.tile([128, 384], f32, name="fo", tag="fo",
                                      bufs=2)
                    for k in range(CT):
                        nc.tensor.matmul(
                            fo[:],
                            aon_sb[k][:, 128 * mt:128 * (mt + 1)],
                            wp_sb[:, C * k + n0:C * k + n1],
                            start=(k == 0), stop=(k == CT - 1))
                    # bias add + PSUM evacuation in one DVE op
                    nc.vector.tensor_add(yst[:, n0:n1], fo[:],
                                         bb_sb[:, n0:n1])
                nc.sync.dma_start(y[128 * mt:128 * (mt + 1), :], yst[:])

    nc.compile()
    return nc


def _get_compiled():
    global _COMPILED
    if _COMPILED is None:
        _COMPILED = _build()
    return _COMPILED


def _run(inputs, trace=False):
    from concourse.bass_utils import run_bass_kernel_spmd

    nc = _get_compiled()
    x = np.asarray(inputs["x"], dtype=np.float32)
    w_qkv = np.ascontiguousarray(np.asarray(inputs["w_qkv"], dtype=np.float32))
    w_proj = np.ascontiguousarray(np.asarray(inputs["w_proj"], dtype=np.float32))
    b_bcast = np.ascontiguousarray(np.broadcast_to(
        np.asarray(inputs["b_proj"], dtype=np.float32).reshape(1, C),
        (128, C)))
    xT_full = np.ascontiguousarray(x[0].T)  # [C, N]
    w_q = np.ascontiguousarray(w_qkv[:, 0:C])
    w_k = np.ascontiguousarray(w_qkv[:, C:2 * C])
    w_v = np.ascontiguousarray(w_qkv[:, 2 * C:3 * C])

    in_maps = []
    for c in range(CORES):
        in_maps.append({
            "xT": np.ascontiguousarray(xT_full[:, S * c:S * (c + 1)]),
            "w_k": w_k,
            "w_v": w_v,
            "w_q": w_q,
            "w_proj": w_proj,
            "b_bcast": b_bcast,
        })
    res = run_bass_kernel_spmd(nc, in_maps, core_ids=list(range(CORES)),
                               trace=trace)
    out = np.concatenate([res.results[c]["y"] for c in range(CORES)], axis=0)
    return out[None, :, :].astype(np.float32), res


def kernel(**inputs) -> np.ndarray:
    out, _ = _run(inputs, trace=False)
    return out


# revision 37
# speedup vs baseline: 1.2448x; 1.0568x over previous
"""Trainium2 Bass kernel: sequence-parallel multi-head self-attention block.

Computes y = proj(softmax(Q K^T / sqrt(D)) V) + b_proj for B=1, N=4096, C=768,
H=12 heads, sharded over 8 NeuronCores by sequence (512 query rows per core).

v2.5 structure (vs the 437us baseline):
  - w_qkv pre-split on the host into K/V/Q column blocks; the K projection
    (feeding the first collective) runs with only xT + half of w_k ahead of
    it on the DMA rings.  Four collectives: K-half1, V-half1, K-half2,
    V-half2, triggered as early as possible.
  - exp(softmax) split across TWO engines: ScalarE exact exp LUT, VectorE
    Schraudolph exp (int16 round of s*SCALE*128/ln2 + bias, bitcast bf16).
  - software pipeline: scores+exp run 2 k-tiles ahead of AV so the
    in-order PE queue never waits on exp.
  - score matmuls for the two heads of a pair occupy different PE row
    groups (64-dim contraction) and run concurrently; AV matmuls occupy
    different column groups (M=64 each into one shared PSUM bank halves)
    and also pack.  Softmax denominators come from M=1 ones-matmuls
    4-way column-packed into a single PSUM bank, opened by a zeroing
    matmul so `start` bank-clear semantics never bite.
  - per-head-pair normalization (reciprocal_approx_fast + rank-1 broadcast
    matmul into both partition halves) is emitted inside the NEXT pair's
    loop so the PE never idles at pair boundaries.
  - gather/weight loads are single multi-dim DMAs; next pair's loads are
    issued from inside the current pair's loop (real prefetch).
"""

import numpy as np

CORES = 8
N = 4096
S = N // CORES          # 512 query rows per core
C = 768
H = 12
D = 64
HP = H // 2             # head-pair partition tiles
CT = C // 128           # 6 contraction tiles over C
KT = N // 128           # 32 key tiles
CH = C // 2
SCALE = float(D) ** -0.5
# Schraudolph exp in bf16-via-int16: exp(s*SCALE) ~ bitcast(int16(round(
#   s*SCALE*128/ln2 + (127*128 - 5.5))))
EXP_A = SCALE * 128.0 / float(np.log(2.0))
EXP_B = 127.0 * 128.0 - 5.5

_COMPILED = None


def _build():
    from contextlib import ExitStack

    import concourse.tile as tile
    from concourse import bacc, mybir

    import ml_dtypes

    f32 = mybir.dt.float32
    f32r = mybir.dt.float32r
    bf16 = mybir.dt.bfloat16
    i16 = mybir.dt.int16
    EXP = mybir.ActivationFunctionType.Exp
    MULT = mybir.AluOpType.mult
    ADD = mybir.AluOpType.add

    nc = bacc.Bacc("TRN2", target_bir_lowering=False, debug=False,
                   num_devices=CORES)

    xT = nc.dram_tensor("xT", [C, S], f32, kind="ExternalInput")
    w_k = nc.dram_tensor("w_k", [C, C], f32, kind="ExternalInput")
    w_v = nc.dram_tensor("w_v", [C, C], f32, kind="ExternalInput")
    w_q = nc.dram_tensor("w_q", [C, C], f32, kind="ExternalInput")
    w_proj = nc.dram_tensor("w_proj", [C, C], f32, kind="ExternalInput")
    b_bcast = nc.dram_tensor("b_bcast", [128, C], f32, kind="ExternalInput")
    y = nc.dram_tensor("y", [S, C], f32, kind="ExternalOutput")

    # merged per-half bounce: rows 0-383 = K^T half, rows 384-895 = V half
    # (V's [S, CH] rows padded to 512-wide so the gather APs stay clean)
    BR = CH + S
    bnc_a = nc.dram_tensor("bnc_a", [BR, S], bf16)
    bnc_b = nc.dram_tensor("bnc_b", [BR, S], bf16)
    gat_a = nc.dram_tensor("gat_a", [CORES * BR, S], bf16,
                           addr_space="Shared")
    gat_b = nc.dram_tensor("gat_b", [CORES * BR, S], bf16,
                           addr_space="Shared")

    # tiny dummy collective fired first: absorbs the ~50us first-collective
    # ncfw warmup while phase 1 computes
    bnc_w = nc.dram_tensor("bnc_w", [1, 16], bf16)
    gat_w = nc.dram_tensor("gat_w", [CORES, 16], bf16, addr_space="Shared")

    groups = [list(range(CORES))]

    def allgather(src, dst):
        nc.gpsimd.collective_compute(
            "AllGather", mybir.AluOpType.bypass, replica_groups=groups,
            ins=[src.ap()], outs=[dst.ap()])

    with tile.TileContext(nc) as tc, ExitStack() as ctx:
        const_pool = ctx.enter_context(tc.tile_pool(name="const", bufs=1))
        qT_pool = ctx.enter_context(tc.tile_pool(name="qT", bufs=1))
        aon_pool = ctx.enter_context(tc.tile_pool(name="aon", bufs=1))
        wp_pool = ctx.enter_context(tc.tile_pool(name="wp", bufs=1))
        nrm_pool = ctx.enter_context(tc.tile_pool(name="nrm", bufs=2))

        ones_dram = nc.inline_tensor(np.ones((128, 128), np.float32),
                                     name="ones_dram")
        ones_dram_bf = nc.inline_tensor(
            np.ones((128, 8), ml_dtypes.bfloat16), name="ones_dram_bf")
        ones_sb = const_pool.tile([128, 128], f32r, name="ones_sb")
        nc.sync.dma_start(ones_sb[:], ones_dram[:, :].bitcast(f32r))
        bb_sb = const_pool.tile([128, C], f32, name="bb_sb")
        nc.sync.dma_start(bb_sb[:], b_bcast[:, :])
        ones_bf = const_pool.tile([128, 1], bf16, name="ones_bf")
        nc.sync.dma_start(ones_bf[:], ones_dram_bf[:, 0:1])
        zeros_bf = const_pool.tile([1, 128], bf16, name="zeros_bf")
        nc.vector.memset(zeros_bf[:], 0.0)

        qT_sb = [qT_pool.tile([128, S], bf16, name=f"qT{m}") for m in range(CT)]
        # normalized attention output per head-pair: [128 dims, S queries]
        aon_sb = [aon_pool.tile([128, S], f32r, name=f"aon{m}")
                  for m in range(CT)]
        wp_sb = wp_pool.tile([128, CT * C], f32r, name="wp_sb")

        # ---- phase 1: local qkv projection + split allgathers ----
        with tc.tile_pool(name="xw", bufs=1) as xw_pool, \
             tc.tile_pool(name="st1", bufs=1) as st1_pool, \
             tc.tile_pool(name="ps1", bufs=1, space="PSUM") as ps1_pool:
            xT_sb = xw_pool.tile([128, CT * S], f32r, name="xTs")
            nc.sync.dma_start(
                xT_sb[:].rearrange("p (k s) -> p k s", s=S),
                xT[:, :].bitcast(f32r).rearrange("(k p) s -> p k s", p=128))
            w_sbs = {}
            for nm in ("k", "v", "q"):
                w_sbs[nm] = xw_pool.tile([128, CT * C], f32r, name=f"w{nm}")
            wk_sb, wv_sb, wq_sb = w_sbs["k"], w_sbs["v"], w_sbs["q"]

            def load_w(w_sb, w_dram, c0, c1):
                # load columns [c0:c1) of every 128-row chunk in one DMA
                nc.sync.dma_start(
                    w_sb[:].rearrange("p (k c) -> p k c", c=C)[:, :, c0:c1],
                    w_dram[:, c0:c1].bitcast(f32r).rearrange(
                        "(k p) c -> p k c", p=128))

            def projT_tile(w_sb, m, dst):
                # dst[128, S] (bf16) = (w[:, 128m:128m+128]^T @ x^T)
                ps = ps1_pool.tile([128, S], f32, name="ps_p",
                                   tag="ps_p", bufs=4)
                for k in range(CT):
                    nc.tensor.matmul(
                        ps[:],
                        w_sb[:, C * k + 128 * m:C * k + 128 * (m + 1)],
                        xT_sb[:, S * k:S * (k + 1)],
                        start=(k == 0), stop=(k == CT - 1))
                nc.scalar.copy(dst[:], ps[:])

            def v_half(h, bnc):
                # V rows in natural [seq, CH] layout for column half h,
                # into bounce rows 384+ (512-wide rows, cols 384: unused)
                n0 = CH * h
                vst = st1_pool.tile([128, 4 * CH], bf16, name="vst",
                                    tag="vst", bufs=2)
                for mt in range(4):
                    ps = ps1_pool.tile([128, CH], f32, name="ps_v",
                                       tag="ps_v", bufs=2)
                    for k in range(CT):
                        nc.tensor.matmul(
                            ps[:],
                            xT_sb[:, S * k + 128 * mt:S * k + 128 * (mt + 1)],
                            wv_sb[:, C * k + n0:C * k + n0 + CH],
                            start=(k == 0), stop=(k == CT - 1))
                    nc.scalar.copy(vst[:, CH * mt:CH * (mt + 1)], ps[:])
                nc.sync.dma_start(
                    bnc[CH:, 0:CH].rearrange("(m p) c -> p m c", p=128),
                    vst[:].rearrange("p (m c) -> p m c", c=CH))

            def k_half(h, bnc):
                kst = st1_pool.tile([128, 3 * S], bf16, name="kst",
                                    tag="kst", bufs=2)
                for i, m in enumerate(range(3 * h, 3 * h + 3)):
                    projT_tile(wk_sb, m, kst[:, S * i:S * (i + 1)])
                nc.sync.dma_start(
                    bnc[0:CH, :].rearrange("(i p) s -> p i s", p=128),
                    kst[:].rearrange("p (i s) -> p i s", s=S))

            load_w(wk_sb, w_k, 0, CH)
            k_half(0, bnc_a)
            load_w(wv_sb, w_v, 0, CH)
            v_half(0, bnc_a)
            allgather(bnc_a, gat_a)
            load_w(wk_sb, w_k, CH, C)
            k_half(1, bnc_b)
            load_w(wv_sb, w_v, CH, C)
            v_half(1, bnc_b)
            allgather(bnc_b, gat_b)
            load_w(wq_sb, w_q, 0, C)
            for m in range(CT):
                projT_tile(wq_sb, m, qT_sb[m])

        # ---- phase 2: attention ----
        with tc.tile_pool(name="kt", bufs=2) as kt_pool, \
             tc.tile_pool(name="vt", bufs=2) as vt_pool, \
             tc.tile_pool(name="pt", bufs=2) as pt_pool, \
             tc.tile_pool(name="sc", bufs=1, space="PSUM") as sc_pool, \
             tc.tile_pool(name="ob", bufs=1, space="PSUM") as ob_pool:

            def load_tiles(hp):
                half = hp // 3
                lhp = hp % 3
                gat_h = (gat_a, gat_b)[half]
                kt = kt_pool.tile([128, N], bf16, name="kt", tag="kt", bufs=2)
                nc.sync.dma_start(
                    kt[:].rearrange("p (r s) -> p r s", s=S),
                    gat_h[:, :].rearrange("(r c) s -> c r s",
                                          c=BR)[128 * lhp:128 * (lhp + 1)])
                vt = vt_pool.tile([128, KT * 128], bf16, name="vt", tag="vt",
                                  bufs=2)
                for tl in range(4):
                    nc.sync.dma_start(
                        vt[:].rearrange("p (r t c) -> p t r c",
                                        t=4, c=128)[:, tl],
                        gat_h[:, :].rearrange("(r v) s -> v r s", v=BR)
                        [CH + 128 * tl:CH + 128 * (tl + 1), :,
                         128 * lhp:128 * (lhp + 1)])
                return kt, vt

            LOOK = 2
            tiles = load_tiles(0)
            pending_norm = None
            for hp in range(HP):
                kt, vt = tiles
                obp = ob_pool.tile([128, S], f32, name="obp", tag="obp",
                                   bufs=1)
                zps = ob_pool.tile([128, S], f32, name="zps", tag="zps",
                                   bufs=1)

                def scores_exp(t):
                    views = []
                    for sub in range(2):
                        sc = sc_pool.tile([128, S], f32, name=f"sc{sub}",
                                          tag=f"sc{sub}", bufs=3)
                        po = 64 * sub
                        nc.tensor.matmul(
                            sc[:],
                            kt[po:po + 64, 128 * t:128 * (t + 1)],
                            qT_sb[hp][po:po + 64, :],
                            start=True, stop=True)
                        if (t + sub) % 2 == 0:
                            pt = pt_pool.tile([128, S], bf16,
                                              name=f"ptb{sub}",
                                              tag=f"ptb{sub}", bufs=4)
                            nc.scalar.activation(pt[:], sc[:], EXP,
                                                 scale=SCALE)
                            views.append(pt[:])
                        else:
                            pt = pt_pool.tile([128, S], i16,
                                              name=f"pti{sub}",
                                              tag=f"pti{sub}", bufs=4)
                            nc.vector.tensor_scalar(pt[:], sc[:],
                                                    EXP_A, EXP_B, MULT, ADD)
                            views.append(pt[:].bitcast(bf16))
                    return views

                def av(t, views):
                    if t == 0:
                        # open both accumulator banks with a zeroing matmul
                        # (sets has_written everywhere; all real matmuls
                        # accumulate with start=False in any order)
                        for bank in (obp, zps):
                            nc.tensor.matmul(bank[:], zeros_bf[:],
                                             qT_sb[hp][0:1, :],
                                             start=True, stop=False,
                                             skip_group_check=True)
                    for sub in range(2):
                        nc.tensor.matmul(
                            obp[64 * sub:64 * (sub + 1), :],
                            vt[:, 128 * t + 64 * sub:128 * t + 64 * (sub + 1)],
                            views[sub],
                            start=False, stop=(t == KT - 1),
                            skip_group_check=True)

                def zmm(tp, views_p, views_c):
                    # 4 column-packed M=1 ones-matmuls: Z accumulators at
                    # partitions 0/32 (head A, tiles tp/tp+1) and 64/96 (B)
                    last = (tp == KT - 2)
                    for idx, vw in enumerate(
                            (views_p[0], views_c[0], views_p[1], views_c[1])):
                        pos = 32 * idx
                        nc.tensor.matmul(
                            zps[pos:pos + 1, :], ones_bf[:], vw,
                            start=False, stop=(last and idx == 3),
                            tile_position=(0, pos), skip_group_check=True)

                views_by_t = {}
                for t in range(KT):
                    views_by_t[t] = scores_exp(t)
                    if t == 6 and hp + 1 < HP:
                        tiles = load_tiles(hp + 1)  # prefetch next pair
                    if t == 10 and hp == 0:
                        # proj weights: needed only in phase 3, keep the
                        # early DMA rings clear for the collectives
                        nc.sync.dma_start(
                            wp_sb[:].rearrange("p (k c) -> p k c", c=C),
                            w_proj[:, :].bitcast(f32r).rearrange(
                                "(k p) c -> p k c", p=128))
                    if t == 8 and pending_norm is not None:
                        pending_norm(
                            lambda: sc_pool.tile([128, S], f32, name="bc",
                                                 tag="sc0", bufs=3))
                        pending_norm = None
                    if t >= LOOK:
                        tv = t - LOOK
                        av(tv, views_by_t[tv])
                        if tv % 2 == 1:
                            zmm(tv - 1, views_by_t[tv - 1], views_by_t[tv])
                            del views_by_t[tv - 1], views_by_t[tv]
                for t in range(KT - LOOK, KT):
                    av(t, views_by_t[t])
                    if t % 2 == 1:
                        zmm(t - 1, views_by_t[t - 1], views_by_t[t])
                        del views_by_t[t - 1], views_by_t[t]

                # evacuate accumulators now (frees PSUM for the next pair);
                # the arithmetic of the normalization is deferred into the
                # next pair's loop (pending_norm) to keep the PE dense
                aoTu = nrm_pool.tile([128, S], f32, name="aoTu", tag="aoTu",
                                     bufs=2)
                zc = nrm_pool.tile([128, S], f32, name="zc", tag="zc", bufs=2)
                nc.scalar.copy(aoTu[:], obp[:])
                nc.scalar.copy(zc[0:97, :], zps[0:97, :])

                def make_norm(hp, aoTu, zc):
                    def norm(bc_alloc):
                        zs = nrm_pool.tile([1, 2 * S], f32, name="zs",
                                           tag="zs", bufs=2)
                        zs2 = nrm_pool.tile([1, 2 * S], f32, name="zs2",
                                            tag="zs2", bufs=2)
                        zsum = nrm_pool.tile([1, 2 * S], f32, name="zsum",
                                             tag="zsum", bufs=2)
                        rz = nrm_pool.tile([1, 2 * S], f32, name="rz",
                                           tag="rz", bufs=2)
                        # gather Z accumulator rows {0,64} and {32,96} into
                        # single-partition vectors, pair-sum, reciprocal
                        # (spread over queues so the 4 issues overlap)
                        nc.scalar.dma_start(zs[0:1, 0:S], zc[0:1, :])
                        nc.gpsimd.dma_start(zs[0:1, S:2 * S], zc[64:65, :])
                        nc.scalar.dma_start(zs2[0:1, 0:S], zc[32:33, :])
                        nc.gpsimd.dma_start(zs2[0:1, S:2 * S], zc[96:97, :])
                        nc.vector.tensor_add(zsum[:], zs[:], zs2[:])
                        nc.vector.reciprocal_approx_fast(rz[:], zsum[:])
                        bc = bc_alloc()
                        nc.tensor.matmul(
                            bc[0:64, :], ones_sb[0:1, 0:64].bitcast(f32),
                            rz[0:1, 0:S], start=True, stop=True,
                            skip_group_check=True)
                        nc.tensor.matmul(
                            bc[64:128, :], ones_sb[0:1, 64:128].bitcast(f32),
                            rz[0:1, S:2 * S], start=True, stop=True,
                            tile_position=(0, 64), skip_group_check=True)
                        nc.vector.tensor_mul(aon_sb[hp][:], aoTu[:], bc[:])
                    return norm

                pending_norm = make_norm(hp, aoTu, zc)

        # ---- phase 3: output projection + bias ----
        # k=0..4 contraction steps of the first units run BEFORE the last
        # pair's normalization so the PE stays busy during its latency
        with tc.tile_pool(name="yst", bufs=2) as y_pool, \
             tc.tile_pool(name="fo", bufs=2, space="PSUM") as fo_pool:
            units = [(mt, n0) for mt in range(S // 128) for n0 in (0, 384)]
            fo_tiles = {}

            def proj_partial(u):
                mt, n0 = units[u]
                fo = fo_pool.tile([128, 384], f32, name="fo", tag="fo",
                                  bufs=4)
                for k in range(CT - 1):
                    nc.tensor.matmul(
                        fo[:],
                        aon_sb[k][:, 128 * mt:128 * (mt + 1)],
                        wp_sb[:, C * k + n0:C * k + n0 + 384],
                        start=(k == 0), stop=False)
                fo_tiles[u] = fo

            for u in range(4):
                proj_partial(u)
            # last pair's 1/Z; its latency overlaps the matmuls above
            pending_norm(
                lambda: fo_pool.tile([128, S], f32, name="bcf",
                                     tag="bcf", bufs=1))
            ysts = {}
            for u in range(len(units)):
                if u not in fo_tiles:
                    proj_partial(u)
                mt, n0 = units[u]
                fo = fo_tiles[u]
                k = CT - 1
                nc.tensor.matmul(
                    fo[:],
                    aon_sb[k][:, 128 * mt:128 * (mt + 1)],
                    wp_sb[:, C * k + n0:C * k + n0 + 384],
                    start=False, stop=True)
                if mt not in ysts:
                    ysts[mt] = y_pool.tile([128, C], f32, name="yst",
                                           tag="yst", bufs=2)
                # bias add + PSUM evacuation in one DVE op
                nc.vector.tensor_add(ysts[mt][:, n0:n0 + 384], fo[:],
                                     bb_sb[:, n0:n0 + 384])
                if n0 == 384:
                    nc.sync.dma_start(y[128 * mt:128 * (mt + 1), :],
                                      ysts[mt][:])

    nc.compile()
    return nc


def _get_compiled():
    global _COMPILED
    if _COMPILED is None:
        _COMPILED = _build()
    return _COMPILED


def _run(inputs, trace=False):
    from concourse.bass_utils import run_bass_kernel_spmd

    nc = _get_compiled()
    x = np.asarray(inputs["x"], dtype=np.float32)
    w_qkv = np.ascontiguousarray(np.asarray(inputs["w_qkv"], dtype=np.float32))
    w_proj = np.ascontiguousarray(np.asarray(inputs["w_proj"], dtype=np.float32))
    b_bcast = np.ascontiguousarray(np.broadcast_to(
        np.asarray(inputs["b_proj"], dtype=np.float32).reshape(1, C),
        (128, C)))
    xT_full = np.ascontiguousarray(x[0].T)  # [C, N]
    w_q = np.ascontiguousarray(w_qkv[:, 0:C])
    w_k = np.ascontiguousarray(w_qkv[:, C:2 * C])
    w_v = np.ascontiguousarray(w_qkv[:, 2 * C:3 * C])

    in_maps = []
    for c in range(CORES):
        in_maps.append({
            "xT": np.ascontiguousarray(xT_full[:, S * c:S * (c + 1)]),
            "w_k": w_k,
            "w_v": w_v,
            "w_q": w_q,
            "w_proj": w_proj,
            "b_bcast": b_bcast,
        })
    res = run_bass_kernel_spmd(nc, in_maps, core_ids=list(range(CORES)),
                               trace=trace)
    out = np.concatenate([res.results[c]["y"] for c in range(CORES)], axis=0)
    return out[None, :, :].astype(np.float32), res


def kernel(**inputs) -> np.ndarray:
    out, _ = _run(inputs, trace=False)
    return out
